# revision 7
# baseline (speedup 1.0000x reference)
"""Trainium2 Bass kernel for nn_CrossScaleFusion (segment mean pooling +
cross-scale attention fusion), SPMD over 8 NeuronCores.

Sharding: core i handles batch b = i//2, frame-half h = i%2 (2048 frames).
Frame-level work (K/V projections for MHA1, all of MHA4, gate4) is split
across the pair; the small beat/bar-level pipeline is computed redundantly
on both cores of a pair after two pairwise AllReduces:
  AR1: beat segment-sum partials, AR2: MHA1 unnormalized attention + softmax
  denominators (exact combine; exp without max-subtraction is safe at these
  score magnitudes).
All matmuls run in bf16 with fp32 PSUM accumulation; the frame residual is
kept in fp32. Everything on-device is in feature-major layout (D on
partitions), which makes per-feature biases cheap; per-token softmax
denominators are broadcast across partitions with a tiny E-matrix matmul.
"""
import math
import sys

for _p in ('/opt/trn_rl_repo',):
    if _p not in sys.path:
        sys.path.insert(0, _p)

import numpy as np
import ml_dtypes

import concourse.bass as bass
import concourse.tile as tile
from concourse import bacc, mybir
from concourse.bass_utils import run_bass_kernel_spmd
from concourse.masks import make_identity

B, T, D, M, MB, POS, NH, K = 4, 4096, 512, 256, 64, 32, 8, 4
HD = D // NH          # 64
TH = T // 2           # 2048 frames per core
NC_ = 8
PAIRS = [[0, 1], [2, 3], [4, 5], [6, 7]]
XW = 65 * NH          # 520, V with per-head ones column
F32 = mybir.dt.float32
BF16 = mybir.dt.bfloat16
P = 128


# ---------------------------------------------------------------------------
# host-side prep
# ---------------------------------------------------------------------------

def _bf(x):
    return np.ascontiguousarray(np.asarray(x, np.float32).astype(ml_dtypes.bfloat16))


def _f32(x):
    return np.ascontiguousarray(np.asarray(x, np.float32))


def _tile_cols(x, p=128):
    """[Kp, N] -> [p, (Kp//p)*N] with row-tile ci at column block ci."""
    kp, n = x.shape
    return np.ascontiguousarray(
        x.reshape(kp // p, p, n).transpose(1, 0, 2).reshape(p, (kp // p) * n))


def _fourier(pos, dim):
    half = dim // 2
    freqs = np.exp(np.linspace(math.log(1.0), math.log(1000.0), half))
    ang = pos[..., None] * freqs
    return np.concatenate([np.sin(ang), np.cos(ang)], axis=-1).astype(np.float32)


def _xlayout_w(wv):
    """[D, D] -> [D, 520]: head h at cols 65h..65h+63, zero col at 65h+64."""
    out = np.zeros((D, XW), np.float32)
    for h in range(NH):
        out[:, 65 * h:65 * h + HD] = wv[:, HD * h:HD * h + HD]
    return out


def _xlayout_bias(bv):
    """[D] -> [128, 520] replicated rows; ones column gets bias 1.0."""
    row = np.zeros((XW,), np.float32)
    for h in range(NH):
        row[65 * h:65 * h + HD] = bv[HD * h:HD * h + HD]
        row[65 * h + HD] = 1.0
    return np.broadcast_to(row, (128, XW)).copy()


def host_prep(inputs):
    """Returns (shared constant map, list of 8 per-core maps)."""
    fe = _f32(inputs['frame_emb'])
    bb = np.asarray(inputs['beat_bounds'])
    s = np.clip(bb[..., 0], 0, T - 1).astype(np.int64)
    e = np.maximum(s + 1, np.minimum(bb[..., 1], T)).astype(np.int64)

    shared = {}
    w1 = _f32(inputs['beat_proj_w'])
    w2 = _f32(inputs['bar_proj_w'])
    pos_b = np.clip(np.arange(M, dtype=np.float32) / (M - 1), 0, 1)
    pos_r = np.clip(np.arange(MB, dtype=np.float32) / (MB - 1), 0, 1)
    ffp1 = _fourier(pos_b, POS) @ w1[D:] + _f32(inputs['beat_proj_b'])  # [M, D]
    ffp2 = _fourier(pos_r, POS) @ w2[D:] + _f32(inputs['bar_proj_b'])  # [MB, D]
    shared['W1'] = _bf(_tile_cols(w1[:D]))
    shared['W2'] = _bf(_tile_cols(w2[:D]))
    shared['ffp1T'] = _f32(_tile_cols(ffp1.T))   # [128, 4*256]
    shared['ffp2T'] = _f32(_tile_cols(ffp2.T))   # [128, 4*64]
    S2 = np.zeros((MB, M), np.float32)
    for r in range(MB):
        S2[r, r * K:min(r * K + K, M)] = 1.0
    shared['S2T'] = _bf(_tile_cols(S2.T))        # [128, 2*64]
    shared['recip_bar'] = _f32((1.0 / S2.sum(1))[:, None])  # [64, 1]
    E8 = np.zeros((8, D), np.float32)
    for h in range(NH):
        E8[h, HD * h:HD * h + HD] = 1.0
    shared['E8'] = _bf(E8)                       # [8, 512]
    for i in range(4):
        wq = _f32(inputs['attn_wq'][i]) / math.sqrt(HD)
        bq = _f32(inputs['attn_bq'][i]) / math.sqrt(HD)
        shared[f'wq{i}'] = _bf(_tile_cols(wq))
        shared[f'wk{i}'] = _bf(_tile_cols(_f32(inputs['attn_wk'][i])))
        shared[f'wo{i}'] = _bf(_tile_cols(_f32(inputs['attn_wo'][i])))
        shared[f'wv{i}'] = _bf(_tile_cols(_xlayout_w(_f32(inputs['attn_wv'][i]))))
        shared[f'bvX{i}'] = _f32(_xlayout_bias(_f32(inputs['attn_bv'][i])))
        shared[f'gw{i}'] = _bf(_tile_cols(_f32(inputs['gate_w'][i])))
        shared[f'bq{i}'] = _f32(bq.reshape(4, 128).T)
        shared[f'bk{i}'] = _f32(_f32(inputs['attn_bk'][i]).reshape(4, 128).T)
        shared[f'bo{i}'] = _f32(_f32(inputs['attn_bo'][i]).reshape(4, 128).T)
        shared[f'gb{i}'] = _f32(_f32(inputs['gate_b'][i]).reshape(4, 128).T)

    per_core = []
    for core in range(NC_):
        b, h = core // 2, core % 2
        fTh = fe[b, h * TH:(h + 1) * TH].T          # [512, 2048]
        Sh = np.zeros((M, TH), np.float32)
        for m in range(M):
            lo = max(int(s[b, m]) - h * TH, 0)
            hi = min(int(e[b, m]) - h * TH, TH)
            if lo < hi:
                Sh[m, lo:hi] = 1.0
        cm = {
            'fT32': _f32(_tile_cols(fTh)),              # [128, 4*2048]
            'fT_bf': _bf(_tile_cols(fTh)),              # [128, 4*2048]
            'f_nat': _bf(_tile_cols(fe[b, h * TH:(h + 1) * TH])),  # [128, 16*512]
            'STs': _bf(_tile_cols(Sh.T)),               # [128, 16*256]
            'recip_len': _f32((1.0 / (e[b] - s[b])).reshape(2, 128).T),  # [128, 2]
        }
        per_core.append(cm)
    return shared, per_core


# ---------------------------------------------------------------------------
# device program
# ---------------------------------------------------------------------------

def build_program():
    nc = bacc.Bacc()

    def param(name, shape, dt=BF16, out=False):
        return nc.declare_dram_parameter(name, list(shape), dt, isOutput=out)

    d = {}
    d['fT32'] = param('fT32', [P, 4 * TH], F32)
    d['fT_bf'] = param('fT_bf', [P, 4 * TH])
    d['f_nat'] = param('f_nat', [P, 16 * D])
    d['STs'] = param('STs', [P, 16 * M])
    d['recip_len'] = param('recip_len', [P, 2], F32)
    d['W1'] = param('W1', [P, 4 * D])
    d['W2'] = param('W2', [P, 4 * D])
    d['ffp1T'] = param('ffp1T', [P, 4 * M], F32)
    d['ffp2T'] = param('ffp2T', [P, 4 * MB], F32)
    d['S2T'] = param('S2T', [P, 2 * MB])
    d['recip_bar'] = param('recip_bar', [MB, 1], F32)
    for i in range(4):
        for nm in ('wq', 'wk', 'wo'):
            d[f'{nm}{i}'] = param(f'{nm}{i}', [P, 4 * D])
        d[f'wv{i}'] = param(f'wv{i}', [P, 4 * XW])
        d[f'gw{i}'] = param(f'gw{i}', [P, 8 * D])
        d[f'bvX{i}'] = param(f'bvX{i}', [P, XW], F32)
        for nm in ('bq', 'bk', 'bo', 'gb'):
            d[f'{nm}{i}'] = param(f'{nm}{i}', [P, 4], F32)
    d['frT'] = param('frT', [P, 4 * TH], F32, out=True)
    d['brT'] = param('brT', [P, 4 * M], F32, out=True)
    d['rfT'] = param('rfT', [P, 4 * MB], F32, out=True)

    with tile.TileContext(nc) as tc:
        _build_body(nc, tc, d)
    nc.compile()
    return nc


def _build_body(nc, tc, d):
    from contextlib import ExitStack
    AF = mybir.ActivationFunctionType
    ALU = mybir.AluOpType
    sync, ve, se, te = nc.sync, nc.vector, nc.scalar, nc.tensor

    root = ExitStack()
    persist = root.enter_context(tc.tile_pool(name="persist", bufs=1))
    wpool = root.enter_context(tc.tile_pool(name="wstream", bufs=3))
    stream = root.enter_context(tc.tile_pool(name="stream", bufs=4))
    psA = root.enter_context(tc.tile_pool(name="psA", bufs=3, space="PSUM"))
    psO = root.enter_context(tc.tile_pool(name="psO", bufs=2, space="PSUM"))
    dram = root.enter_context(tc.tile_pool(name="drambb", bufs=1, space="DRAM"))

    def dma(dst, src):
        sync.dma_start(dst, src)

    def wtile(name):
        cols = {'wv': 4 * XW, 'gw': 8 * D}.get(name[:2], 4 * D)
        t = wpool.tile([P, cols], BF16, tag="w")
        dma(t[:], d[name][:])
        return t

    def btile(name, pl):
        t = pl.tile([P, 4], F32, tag=f"b_{name}")
        dma(t[:], d[name][:])
        return t

    def mm(out, lhsT, rhs, start, stop):
        te.matmul(out, lhsT, rhs, start=start, stop=stop, skip_group_check=True)

    # ---- persistent tiles -------------------------------------------------
    fTbf = persist.tile([P, 4 * TH], BF16, tag="fTbf")
    dma(fTbf[:], d['fT_bf'][:])
    ident = persist.tile([P, P], BF16, tag="ident")
    make_identity(nc, ident)
    ones64 = persist.tile([1, HD], BF16, tag="ones64")
    ve.memset(ones64[:], 1.0)
    bT = persist.tile([P, 4 * M], BF16, tag="bT")
    rT = persist.tile([P, 4 * MB], BF16, tag="rT")
    bfT = persist.tile([P, 4 * M], BF16, tag="bfT")
    rfT = persist.tile([P, 4 * MB], BF16, tag="rfT")
    brT = persist.tile([P, 4 * M], BF16, tag="brT")

    # ---- generic feature-major projection --------------------------------
    def fm_proj(out_sb, out_L, w_sb, rhs_sb, rhs_L, Kc=4, bias=None, act=None,
                extra=None, chunk=512, out_dt=BF16):
        for dj in range(4):
            for c0 in range(0, out_L, chunk):
                cw = min(chunk, out_L - c0)
                ps = psA.tile([P, cw], F32, tag="ps")
                for ci in range(Kc):
                    mm(ps[:], w_sb[:, ci * D + dj * P: ci * D + (dj + 1) * P],
                       rhs_sb[:, ci * rhs_L + c0: ci * rhs_L + c0 + cw],
                       start=(ci == 0), stop=(ci == Kc - 1))
                dst = out_sb[:, dj * out_L + c0: dj * out_L + c0 + cw]
                if act is not None:
                    se.activation(dst, ps[:], act,
                                  bias=bias[:, dj:dj + 1] if bias is not None else 0.0)
                elif bias is not None:
                    ve.tensor_scalar_add(dst, ps[:], bias[:, dj:dj + 1])
                elif extra is not None:
                    ve.tensor_add(dst, ps[:], extra[:, dj * out_L + c0: dj * out_L + c0 + cw])
                else:
                    ve.tensor_copy(dst, ps[:])

    def attention(qT, Lq, vX, Lk, kT, attn_u, l_sb):
        """Unnormalized attention, feature-major. attn_u [128, 4*Lq], l_sb [8, Lq]."""
        ktiles = max(1, Lk // P)
        kpart = min(Lk, P)
        for h in range(NH):
            r0 = HD * (h % 2)
            cb = (h // 2) * Lk
            qb = (h // 2) * Lq
            for c0 in range(0, Lq, 512):
                cw = min(512, Lq - c0)
                pso = psO.tile([65, cw], F32, tag="ps_av")
                for k in range(ktiles):
                    ps = psA.tile([kpart, cw], F32, tag="ps")
                    mm(ps[:], kT[r0:r0 + HD, cb + k * P: cb + k * P + kpart],
                       qT[r0:r0 + HD, qb + c0: qb + c0 + cw], start=True, stop=True)
                    ptile = stream.tile([P, cw], BF16, tag="pt_exp")
                    se.activation(ptile[:kpart, :], ps[:], AF.Exp)
                    mm(pso[:], vX[:kpart, k * XW + 65 * h: k * XW + 65 * h + 65],
                       ptile[:kpart, :], start=(k == 0), stop=(k == ktiles - 1))
                ve.tensor_copy(attn_u[r0:r0 + HD, qb + c0: qb + c0 + cw], pso[0:HD, :])
                ve.tensor_copy(l_sb[0:1, h * Lq + c0: h * Lq + c0 + cw],
                               pso[HD:HD + 1, :])

    def normalize(attn_u, l_bf, Lq, attn_n=None):
        """attn_n = attn_u / l broadcast; in-place when attn_n is None.
        l_bf: [1, 8*Lq] bf16 head-major denominators."""
        if attn_n is None:
            attn_n = attn_u
        for dj in range(4):
            for c0 in range(0, Lq, 512):
                cw = min(512, Lq - c0)
                ps = psA.tile([P, cw], F32, tag="ps", name="ps_lrep")
                for half in (0, 1):
                    h = 2 * dj + half
                    mm(ps[HD * half:HD * half + HD, :], ones64[0:1, :],
                       l_bf[0:1, h * Lq + c0: h * Lq + c0 + cw],
                       start=True, stop=True)
                lr = stream.tile([P, cw], BF16, tag="lrep")
                with nc.allow_low_precision(reason="1/l feeds bf16 mul"):
                    ve.reciprocal(lr[:], ps[:])
                ve.tensor_tensor(attn_n[:, dj * Lq + c0: dj * Lq + c0 + cw],
                                 attn_u[:, dj * Lq + c0: dj * Lq + c0 + cw],
                                 lr[:], op=ALU.mult)

    def vproj(vX_sb, src_fm, Lk, wv_sb, bvX_sb, ntiles):
        for t in range(ntiles):
            tp = min(P, Lk)
            ps = psA.tile([tp, XW], F32, tag="ps")
            for ci in range(4):
                for c0, cw in ((0, 512), (512, 8)):
                    mm(ps[:, c0:c0 + cw],
                       src_fm[:, ci * Lk + t * P: ci * Lk + t * P + tp],
                       wv_sb[:, ci * XW + c0: ci * XW + c0 + cw],
                       start=(ci == 0), stop=(ci == 3))
            ve.tensor_add(vX_sb[:tp, t * XW:(t + 1) * XW], ps[:], bvX_sb[:tp, :])

    def gate(x_fm, y_fm, Lq, gw_sb, gb_sb, out_bf, out_f32=None, out_dram=None):
        """out = x + sigmoid([x|y] @ gw + gb) * y, feature-major."""
        for dj in range(4):
            for c0 in range(0, Lq, 512):
                cw = min(512, Lq - c0)
                ps = psA.tile([P, cw], F32, tag="ps")
                for ci in range(8):
                    src = x_fm if ci < 4 else y_fm
                    cc = ci % 4
                    mm(ps[:], gw_sb[:, ci * D + dj * P: ci * D + dj * P + P],
                       src[:, cc * Lq + c0: cc * Lq + c0 + cw],
                       start=(ci == 0), stop=(ci == 7))
                gt = stream.tile([P, cw], BF16, tag="gt")
                se.activation(gt[:], ps[:], AF.Sigmoid, bias=gb_sb[:, dj:dj + 1])
                sl = slice(dj * Lq + c0, dj * Lq + c0 + cw)
                if out_f32 is not None:
                    prod = stream.tile([P, cw], F32, tag="gprod")
                    ve.tensor_tensor(prod[:], gt[:], y_fm[:, sl], op=ALU.mult)
                    ve.tensor_add(out_f32[:, sl], x_fm[:, sl], prod[:])
                    ve.tensor_copy(out_bf[:, sl], out_f32[:, sl])
                    if out_dram is not None:
                        dma(out_dram[:, sl], out_f32[:, sl])
                else:
                    prod = stream.tile([P, cw], BF16, tag="gprodb")
                    ve.tensor_tensor(prod[:], gt[:], y_fm[:, sl], op=ALU.mult)
                    ve.tensor_add(out_bf[:, sl], x_fm[:, sl], prod[:])

    # =======================================================================
    # Stage A+B: beat pooling -> AR1 -> beat proj; bar pooling + proj
    # =======================================================================
    with ExitStack() as st:
        early = st.enter_context(tc.tile_pool(name="early", bufs=1))
        ar1_in = dram.tile([P, 2 * D], F32, tag="ar1i")
        ar1_out = dram.tile([P, 2 * D], F32, tag="ar1o")
        STs = early.tile([P, 16 * M], BF16, tag="STs")
        dma(STs[:], d['STs'][:])
        bsum_sb = early.tile([P, 2 * D], F32, tag="bsum")
        psb = [psA.tile([P, D], F32, tag="ps", name=f"psb{mt}") for mt in range(2)]
        for k in range(16):
            fn = stream.tile([P, D], BF16, tag="fnat")
            dma(fn[:], d['f_nat'][:, k * D:(k + 1) * D])
            for mt in range(2):
                mm(psb[mt][:], STs[:, k * M + mt * P: k * M + (mt + 1) * P], fn[:],
                   start=(k == 0), stop=(k == 15))
        for mt in range(2):
            ve.tensor_copy(bsum_sb[:, mt * D:(mt + 1) * D], psb[mt][:])
        dma(ar1_in[:], bsum_sb[:])
        nc.gpsimd.collective_compute(
            "AllReduce", ALU.add, replica_groups=PAIRS,
            ins=[ar1_in.opt()], outs=[ar1_out.opt()])

        bsumr = early.tile([P, 2 * D], F32, tag="bsumr")
        dma(bsumr[:], ar1_out[:])
        rlen = early.tile([P, 2], F32, tag="rlen")
        dma(rlen[:], d['recip_len'][:])
        bnat = early.tile([P, 2 * D], BF16, tag="bnat")
        for mt in range(2):
            ve.tensor_scalar_mul(bnat[:, mt * D:(mt + 1) * D],
                                 bsumr[:, mt * D:(mt + 1) * D], rlen[:, mt:mt + 1])
        beatT = early.tile([P, 4 * M], BF16, tag="beatT")
        for mt in range(2):
            for dj in range(4):
                pt = psA.tile([P, P], BF16, tag="ps")
                te.transpose(pt[:], bnat[:, mt * D + dj * P: mt * D + (dj + 1) * P], ident[:])
                ve.tensor_copy(beatT[:, dj * M + mt * P: dj * M + (mt + 1) * P], pt[:])
        ffp1 = early.tile([P, 4 * M], F32, tag="ffp1")
        dma(ffp1[:], d['ffp1T'][:])
        fm_proj(bT, M, wtile('W1'), beatT, M, extra=ffp1, chunk=M)

        bpnat = early.tile([P, 2 * D], BF16, tag="bpnat")
        for mt in range(2):
            for dj in range(4):
                pt = psA.tile([P, P], BF16, tag="ps")
                te.transpose(pt[:], bT[:, dj * M + mt * P: dj * M + (mt + 1) * P], ident[:])
                ve.tensor_copy(bpnat[:, mt * D + dj * P: mt * D + (dj + 1) * P], pt[:])
        S2T = early.tile([P, 2 * MB], BF16, tag="S2T")
        dma(S2T[:], d['S2T'][:])
        rbar = early.tile([MB, 1], F32, tag="rbar")
        dma(rbar[:], d['recip_bar'][:])
        ps_bar = psA.tile([MB, D], F32, tag="ps")
        for mt in range(2):
            mm(ps_bar[:], S2T[:, mt * MB:(mt + 1) * MB], bpnat[:, mt * D:(mt + 1) * D],
               start=(mt == 0), stop=(mt == 1))
        barnat = early.tile([MB, D], BF16, tag="barnat")
        ve.tensor_scalar_mul(barnat[:], ps_bar[:], rbar[:])
        barT = early.tile([P, 4 * MB], BF16, tag="barT")
        for dj in range(4):
            pt = psA.tile([P, MB], BF16, tag="ps")
            te.transpose(pt[:, :MB], barnat[:, dj * P:(dj + 1) * P], ident[:MB, :MB])
            ve.tensor_copy(barT[:, dj * MB:(dj + 1) * MB], pt[:, :MB])
        ffp2 = early.tile([P, 4 * MB], F32, tag="ffp2")
        dma(ffp2[:], d['ffp2T'][:])
        fm_proj(rT, MB, wtile('W2'), barT, MB, extra=ffp2, chunk=MB)

    # =======================================================================
    # Stage C: MHA1 beat <- frames (keys split across pair) + AR2 + gate1
    # =======================================================================
    with ExitStack() as st:
        mha1 = st.enter_context(tc.tile_pool(name="mha1", bufs=1))
        q1T = mha1.tile([P, 4 * M], BF16, tag="q1T")
        fm_proj(q1T, M, wtile('wq0'), bT, M, bias=btile('bq0', mha1), chunk=M)
        k1T = mha1.tile([P, 4 * TH], BF16, tag="k1T")
        fm_proj(k1T, TH, wtile('wk0'), fTbf, TH, bias=btile('bk0', mha1))
        v1X = mha1.tile([P, 16 * XW], BF16, tag="v1X")
        bvX0 = mha1.tile([P, XW], F32, tag="bvX0")
        dma(bvX0[:], d['bvX0'][:])
        vproj(v1X, fTbf, TH, wtile('wv0'), bvX0, 16)

        au1 = mha1.tile([P, 4 * M], F32, tag="au1")
        l1 = mha1.tile([1, NH * M], F32, tag="l1")
        attention(q1T, M, v1X, TH, k1T, au1, l1)

        ar2a_in = dram.tile([P, 4 * M], F32, tag="ar2ai")
        ar2a_out = dram.tile([P, 4 * M], F32, tag="ar2ao")
        ar2b_in = dram.tile([1, NH * M], F32, tag="ar2bi")
        ar2b_out = dram.tile([1, NH * M], F32, tag="ar2bo")
        dma(ar2a_in[:], au1[:])
        dma(ar2b_in[:], l1[:])
        nc.gpsimd.collective_compute(
            "AllReduce", ALU.add, replica_groups=PAIRS,
            ins=[ar2a_in.opt()], outs=[ar2a_out.opt()])
        nc.gpsimd.collective_compute(
            "AllReduce", ALU.add, replica_groups=PAIRS,
            ins=[ar2b_in.opt()], outs=[ar2b_out.opt()])
        au1r = mha1.tile([P, 4 * M], F32, tag="au1r")
        l1r = mha1.tile([1, NH * M], F32, tag="l1r")
        dma(au1r[:], ar2a_out[:])
        dma(l1r[:], ar2b_out[:])
        l1rb = mha1.tile([1, NH * M], BF16, tag="l1rb")
        ve.tensor_copy(l1rb[:], l1r[:])

        an1 = mha1.tile([P, 4 * M], BF16, tag="an1")
        normalize(au1r, l1rb, M, an1)
        bffT = mha1.tile([P, 4 * M], BF16, tag="bffT")
        fm_proj(bffT, M, wtile('wo0'), an1, M, bias=btile('bo0', mha1), chunk=M)
        gate(bT, bffT, M, wtile('gw0'), btile('gb0', mha1), bfT)

    # =======================================================================
    # Stage E/F: MHA2 (bar <- beat_fused) and MHA3 (beat <- bar)
    # =======================================================================
    with ExitStack() as st:
        mid = st.enter_context(tc.tile_pool(name="mid", bufs=1))
        q2T = mid.tile([P, 4 * MB], BF16, tag="q2T")
        fm_proj(q2T, MB, wtile('wq1'), rT, MB, bias=btile('bq1', mid), chunk=MB)
        k2T = mid.tile([P, 4 * M], BF16, tag="k2T")
        fm_proj(k2T, M, wtile('wk1'), bfT, M, bias=btile('bk1', mid), chunk=M)
        v2X = mid.tile([P, 2 * XW], BF16, tag="v2X")
        bvX1 = mid.tile([P, XW], F32, tag="bvX1")
        dma(bvX1[:], d['bvX1'][:])
        vproj(v2X, bfT, M, wtile('wv1'), bvX1, 2)
        au2 = mid.tile([P, 4 * MB], BF16, tag="au2")
        l2 = mid.tile([1, NH * MB], BF16, tag="l2")
        attention(q2T, MB, v2X, M, k2T, au2, l2)
        normalize(au2, l2, MB)
        bfb2 = mid.tile([P, 4 * MB], BF16, tag="bfb2")
        fm_proj(bfb2, MB, wtile('wo1'), au2, MB, bias=btile('bo1', mid), chunk=MB)
        rf32 = mid.tile([P, 4 * MB], F32, tag="rf32")
        gate(rT, bfb2, MB, wtile('gw1'), btile('gb1', mid), rfT, out_f32=rf32,
             out_dram=d['rfT'])

        q3T = mid.tile([P, 4 * M], BF16, tag="q3T")
        fm_proj(q3T, M, wtile('wq2'), bfT, M, bias=btile('bq2', mid), chunk=M)
        k3T = mid.tile([P, 4 * MB], BF16, tag="k3T")
        fm_proj(k3T, MB, wtile('wk2'), rfT, MB, bias=btile('bk2', mid), chunk=MB)
        v3X = mid.tile([MB, XW], BF16, tag="v3X")
        bvX2 = mid.tile([P, XW], F32, tag="bvX2")
        dma(bvX2[:], d['bvX2'][:])
        vproj(v3X, rfT, MB, wtile('wv2'), bvX2, 1)
        au3 = mid.tile([P, 4 * M], BF16, tag="au3")
        l3 = mid.tile([1, NH * M], BF16, tag="l3")
        attention(q3T, M, v3X, MB, k3T, au3, l3)
        normalize(au3, l3, M)
        btb = mid.tile([P, 4 * M], BF16, tag="btb")
        fm_proj(btb, M, wtile('wo2'), au3, M, bias=btile('bo2', mid), chunk=M)
        br32 = mid.tile([P, 4 * M], F32, tag="br32")
        gate(bfT, btb, M, wtile('gw2'), btile('gb2', mid), brT, out_f32=br32,
             out_dram=d['brT'])

    # =======================================================================
    # Stage G: MHA4 frames <- beat_refined ; gate4 ; residual ; output
    # =======================================================================
    with ExitStack() as st:
        mha4 = st.enter_context(tc.tile_pool(name="mha4", bufs=1))
        q4T = mha4.tile([P, 4 * TH], BF16, tag="q4T")
        fm_proj(q4T, TH, wtile('wq3'), fTbf, TH, bias=btile('bq3', mha4))
        k4T = mha4.tile([P, 4 * M], BF16, tag="k4T")
        fm_proj(k4T, M, wtile('wk3'), brT, M, bias=btile('bk3', mha4), chunk=M)
        v4X = mha4.tile([P, 2 * XW], BF16, tag="v4X")
        bvX3 = mha4.tile([P, XW], F32, tag="bvX3")
        dma(bvX3[:], d['bvX3'][:])
        vproj(v4X, brT, M, wtile('wv3'), bvX3, 2)
        au4 = mha4.tile([P, 4 * TH], BF16, tag="au4")
        l4 = mha4.tile([1, NH * TH], BF16, tag="l4")
        attention(q4T, TH, v4X, M, k4T, au4, l4)
        normalize(au4, l4, TH)
        o4T = mha4.tile([P, 4 * TH], BF16, tag="o4T")
        fm_proj(o4T, TH, wtile('wo3'), au4, TH, bias=btile('bo3', mha4))
        gw3 = wtile('gw3')
        gb3 = btile('gb3', mha4)
        for dj in range(4):
            for c0 in range(0, TH, 512):
                ps = psA.tile([P, 512], F32, tag="ps")
                for ci in range(8):
                    src = fTbf if ci < 4 else o4T
                    cc = ci % 4
                    mm(ps[:], gw3[:, ci * D + dj * P: ci * D + dj * P + P],
                       src[:, cc * TH + c0: cc * TH + c0 + 512],
                       start=(ci == 0), stop=(ci == 7))
                gt = stream.tile([P, 512], BF16, tag="gt")
                se.activation(gt[:], ps[:], AF.Sigmoid, bias=gb3[:, dj:dj + 1])
                prod = stream.tile([P, 512], F32, tag="gprod")
                ve.tensor_tensor(prod[:], gt[:], o4T[:, dj * TH + c0: dj * TH + c0 + 512],
                                 op=ALU.mult)
                f32t = stream.tile([P, 512], F32, tag="f32t")
                dma(f32t[:], d['fT32'][:, dj * TH + c0: dj * TH + c0 + 512])
                out_t = stream.tile([P, 512], F32, tag="out_t")
                ve.tensor_add(out_t[:], f32t[:], prod[:])
                dma(d['frT'][:, dj * TH + c0: dj * TH + c0 + 512], out_t[:])

    root.close()


# ---------------------------------------------------------------------------
# public entry point
# ---------------------------------------------------------------------------

_NC_CACHE = None


def _get_program():
    global _NC_CACHE
    if _NC_CACHE is None:
        _NC_CACHE = build_program()
    return _NC_CACHE


def _untile_cols(x, ktiles):
    """[128, ktiles*N] -> [ktiles*128, N]"""
    p, tn = x.shape
    n = tn // ktiles
    return x.reshape(p, ktiles, n).transpose(1, 0, 2).reshape(ktiles * p, n)


def kernel(**inputs):
    nc = _get_program()
    shared, per_core = host_prep(inputs)
    in_maps = []
    for c in range(NC_):
        m = dict(shared)
        m.update(per_core[c])
        in_maps.append(m)
    res = run_bass_kernel_spmd(nc, in_maps, list(range(NC_)))
    frame_refined = np.zeros((B, T, D), np.float32)
    beat_refined = np.zeros((B, M, D), np.float32)
    bar_fused = np.zeros((B, MB, D), np.float32)
    for c in range(NC_):
        b, h = c // 2, c % 2
        r = res.results[c]
        frame_refined[b, h * TH:(h + 1) * TH] = _untile_cols(r['frT'], 4).T
        if h == 0:
            beat_refined[b] = _untile_cols(r['brT'], 4).T
            bar_fused[b] = _untile_cols(r['rfT'], 4).T
    return (frame_refined, beat_refined, bar_fused)


if __name__ == "__main__":
    import reference as ref
    inputs = ref.setup_inputs()
    got = kernel(**{k: np.asarray(v) for k, v in inputs.items()})
    exp = ref.reference(**inputs)
    for n, g, e in zip(['frame', 'beat', 'bar'], got, exp):
        e = np.asarray(e)
        err = np.abs(g - e).max() / np.abs(e).max()
        print(n, "max-relerr:", err)
